# revision 19
# baseline (speedup 1.0000x reference)
"""Trainium2 Bass kernel for an 8-batch dense transformer block.

Reference computation (B=8, S=2048, E=1024, H=4096):
    Q = x@Wq + bq; K = x@Wk + bk; V = x@Wv + bv
    attn = softmax(mask(Q K^T) / sqrt(E))
    ctx  = attn @ LN1(V)
    h    = LN2(ctx)
    h    = relu(h@W1 + b1); h = relu(h@W2 + b2)
    out  = V + h

Strategy: pure data parallelism — one batch element per NeuronCore, weights
replicated, no collectives.  Host-side exact algebra folds:
  * scores = (x A) x^T with A = Wq Wk^T / sqrt(E)  (query/key row-bias terms
    are softmax-invariant; the key-column bias term is x (Wk bq)/sqrt(E),
    shipped separately when nonzero)
  * ln2_g/ln2_b folded into W1/b1
  * softmax denominator folded into the LN2 scalars (LN2 sees an exact
    rescale of the unnormalized attention output)
Scores are computed TRANSPOSED (stationary = keys, moving = queries), so the
attention-probability matrix comes out of the PE already key-major and no
P-transpose is needed; the softmax denominator l is recovered through a
constant ones-column appended to LN1(V) during the ctx matmul.
The ctx matmul (P~ @ LN1(V)) runs in fp8e4m3 with DoubleRow perf mode
(2 contraction slabs per instruction, 2x MAC rate); the probabilities are
quantized as P*2^8 and LN1(V) as Vn*2^3, with all scales folded exactly into
the activation-engine scale/bias parameters.  All other matmuls are bf16
(fp32 PSUM accumulation); norms/softmax in fp32.
"""

import os
import sys

if "/opt/trn_rl_repo" not in sys.path:
    sys.path.insert(0, "/opt/trn_rl_repo")

import numpy as np
import ml_dtypes

import concourse.bass as bass
import concourse.tile as tile
from concourse import mybir
from concourse.masks import make_identity

F32 = mybir.dt.float32
BF16 = mybir.dt.bfloat16
E4 = mybir.dt.float8e4

B, S, E, H = 8, 2048, 1024, 4096
SB = S // 128       # 16 token blocks
ET = E // 128       # 8 e tiles
HT = H // 128       # 32 h tiles
KC = S // 512       # 4 key/query chunks
EC = E // 512       # 2 feature chunks
EE = E + 16         # vn free size incl. ones-column (16B-aligned slab stride for DR)
EPS = 1e-5

CTX_E4 = True       # ctx matmul in fp8 DoubleRow
C_OFF = 7.0         # softmax shift:  P~ = exp(s - C)
PSH = 8             # exp output extra scale: P tile holds P~ * 2^8
LN_P = float(np.log(2.0 ** PSH))

LAST_EXEC_TIME_NS = None
LAST_RESULTS = None


# ---------------------------------------------------------------------------
# Workarounds: walrus here rejects >1 embedded sync-wait per instruction.
# ---------------------------------------------------------------------------
def _apply_patches():
    import bass_rust
    import concourse.tile as tile_mod
    from concourse.vector_clock import ScopedClock

    def _patched_drain_and_barrier(self, tick_clock, wait_clock):
        nc = self.nc
        drain_inst = nc.sync.drain()
        wait_clock.add_sem_waits(
            drain_inst.ins, ScopedClock({None: tick_clock.global_clock})
        )
        si = drain_inst.ins.sync_info
        waits = list(si.on_wait)
        drain_inst.ins.sync_info = bass_rust.SyncInfo(
            on_wait=[], on_update=list(si.on_update)
        )
        for w in waits:
            nop = nc.sync.nop(nofuse=True)
            nop.ins.sync_info = bass_rust.SyncInfo(on_wait=[w], on_update=[])
        nc.all_engine_barrier()
        assert self.sems is not None
        popped = nc._tile_sem_poison_stack.pop()
        assert popped is self._sem_poison
        nc.clear_and_free_semaphores(list(self.sems.allocated().values()))
        nc.all_engine_barrier()

    tile_mod.TileContext._drain_and_barrier = _patched_drain_and_barrier


def _fixup_waits(nc, max_waits=1):
    """Hoist excess embedded sync-waits onto NOPs preceding the instruction
    in its engine's program order."""
    import bass_rust

    n_fixed = 0
    for f in nc.m.functions:
        for bb in f.blocks:
            il = list(bb.instructions)
            out = []
            changed = False
            for inst in il:
                si = getattr(inst, "sync_info", None)
                waits = list(si.on_wait) if si is not None else []
                if len(waits) > max_waits:
                    keep = waits[:max_waits]
                    extra = waits[max_waits:]
                    for i, w in enumerate(extra):
                        nop = mybir.InstNoOp(
                            name=f"{inst.name}-waitfix-{i}",
                            sync_info=mybir.SyncInfo(on_wait=[w], on_update=[]),
                            bass_nofuse=True,
                            engine=inst.engine,
                        )
                        out.append(nop)
                    inst.sync_info = bass_rust.SyncInfo(
                        on_wait=keep, on_update=list(si.on_update)
                    )
                    changed = True
                    n_fixed += 1
                out.append(inst)
            if changed:
                bb.instructions = out
    return n_fixed


def _maybe_install_ntff_hook():
    """When tracing is requested, register the axon NTFF profile hook that
    the image's antenv lacks."""
    try:
        import types

        if "antenv.axon_hooks" in sys.modules:
            return
        from trn_agent_boot.trn_boot import _ntff_profile_via_ctypes

        hook = _ntff_profile_via_ctypes("/opt/axon/libaxon_pjrt.so")
        mod = types.ModuleType("antenv.axon_hooks")
        state = {"hook": hook}
        mod.set_axon_ntff_profile_hook = lambda h: state.__setitem__("hook", h)
        mod.get_axon_ntff_profile_hook = lambda: state["hook"]
        sys.modules["antenv.axon_hooks"] = mod
        import antenv

        antenv.axon_hooks = mod
    except Exception:
        pass


# ---------------------------------------------------------------------------
# Device graph
# ---------------------------------------------------------------------------
def _build(flags):
    """Build the per-core Bass graph. flags: has_colbias, has_vbias,
    has_ln1_affine, has_b2."""
    nc = bass.Bass(num_devices=8)

    VN_DT = E4 if CTX_E4 else BF16
    P_DT = E4 if CTX_E4 else BF16
    vnscale = 8.0 if CTX_E4 else 1.0
    c_ones = 8.0                       # ones-column value (exact in e4/bf16)
    hscale = 1.0                       # h leaves LN2 at natural scale (bf16 FFN)
    ln_scale = (c_ones / vnscale) ** 2
    rstd_bias = float(np.log(hscale * c_ones / vnscale))  # 0 when folds cancel

    xb = nc.declare_dram_parameter("xb", [E, S], BF16, isOutput=False)
    a_w = nc.declare_dram_parameter("a_w", [ET, 128, ET, 128], BF16, isOutput=False)
    wv_w = nc.declare_dram_parameter("wv_w", [128, ET, E], BF16, isOutput=False)
    w1_w = nc.declare_dram_parameter("w1_w", [HT, 128, ET, 128], BF16, isOutput=False)
    w2_w = nc.declare_dram_parameter("w2_w", [H, E], BF16, isOutput=False)
    b1_w = nc.declare_dram_parameter("b1_w", [128, HT], F32, isOutput=False)
    if flags["has_colbias"]:
        # per-key exp bias panel: cb + ln(2^PSH) - C, key-major [128, SB]
        cb_w = nc.declare_dram_parameter("cb_w", [128, SB], F32, isOutput=False)
    if flags["has_vbias"]:
        bv_w = nc.declare_dram_parameter("bv_w", [1, E], F32, isOutput=False)
    if flags["has_ln1_affine"]:
        g1_w = nc.declare_dram_parameter("g1_w", [1, E], F32, isOutput=False)
        c1_w = nc.declare_dram_parameter("c1_w", [1, E], F32, isOutput=False)
    if flags["has_b2"]:
        b2_w = nc.declare_dram_parameter("b2_w", [1, E], F32, isOutput=False)
    out_w = nc.declare_dram_parameter("out", [S, E], F32, isOutput=True)

    vscr = nc.dram_tensor("vscr", [SB, 128, E], F32)

    Exp = mybir.ActivationFunctionType.Exp
    Relu = mybir.ActivationFunctionType.Relu
    Ln = mybir.ActivationFunctionType.Ln
    SUB = mybir.AluOpType.subtract
    MUL = mybir.AluOpType.mult

    def dr_pairs(n):
        return range(0, n, 2)

    with tile.TileContext(nc) as tc:
        import contextlib

        stack = contextlib.ExitStack()
        with stack:
            const = stack.enter_context(tc.tile_pool(name="const", bufs=1))
            ident = const.tile([128, 128], BF16)
            make_identity(nc, ident[:])
            eps_t = const.tile([128, 1], F32)
            nc.vector.memset(eps_t[:], EPS)
            expb = const.tile([128, 1], F32)       # exp bias: ln(2^PSH) - C
            nc.vector.memset(expb[:], LN_P - C_OFF)
            ln8_t = const.tile([128, 1], F32)      # ln(8) bias for scale folds
            nc.vector.memset(ln8_t[:], float(np.log(8.0)))
            b1_sb = const.tile([128, HT], F32)
            nc.sync.dma_start(b1_sb[:], b1_w[:])
            if flags["has_colbias"]:
                cb_sb = const.tile([128, SB], F32)
                nc.sync.dma_start(cb_sb[:], cb_w[:])
            if flags["has_vbias"]:
                bv_sb = const.tile([128, E], F32)
                nc.sync.dma_start(bv_sb[:], bv_w[:].broadcast_to([128, E]))
            if flags["has_ln1_affine"]:
                g1_sb = const.tile([128, E], F32)
                nc.sync.dma_start(g1_sb[:], g1_w[:].broadcast_to([128, E]))
                c1_sb = const.tile([128, E], F32)
                nc.sync.dma_start(c1_sb[:], c1_w[:].broadcast_to([128, E]))
            if flags["has_b2"]:
                b2_sb = const.tile([128, E], F32)
                nc.sync.dma_start(b2_sb[:], b2_w[:].broadcast_to([128, E]))

            # Long-lived activations. Stack order matters: hT lives through
            # FFN1; vn/xT/qT are released after phase 2 so the FFN phase can
            # reuse their SBUF.
            acts_ht = stack.enter_context(tc.tile_pool(name="acts_ht", bufs=1))
            hT_parts = [                            # LN2(ctx)^T (feature-major)
                acts_ht.tile([128, ET, 512], BF16, name=f"hT{p}")
                for p in range(KC)
            ]
            fastpath = not any(flags.values())
            n_boot = 6 if fastpath else 0
            if fastpath:
                w1boot = stack.enter_context(tc.tile_pool(name="w1boot", bufs=1))
                w1b_sb = w1boot.tile([128, 6, ET, 128], BF16)
                for hb in range(6):
                    nc.sync.dma_start(w1b_sb[:, hb, :, :], w1_w[hb])
                h1boot_pool = stack.enter_context(tc.tile_pool(name="h1boot", bufs=1))
                h1T_boot = h1boot_pool.tile([128, HT, 512], BF16)
            acts_vn_cm = tc.tile_pool(name="acts_vn", bufs=1)
            acts_vn = acts_vn_cm.__enter__()
            vn = acts_vn.tile([128, SB, EE], VN_DT)  # LN1(V)*vnscale + ones col
            acts_xq_cm = tc.tile_pool(name="acts_xq", bufs=1)
            acts_xq = acts_xq_cm.__enter__()
            xTs = [
                acts_xq.tile([128, S], BF16, name=f"xT{et}") for et in range(ET)
            ]                                       # x^T  (feature-major)
            qT = acts_xq.tile([128, ET, S], BF16)   # (xA)^T

            # ones-column init: cols E..E+8 zero, col E = c_ones
            nc.vector.memset(vn[:, :, E:EE], 0.0)
            nc.vector.memset(vn[:, :, E : E + 1], c_ones)

            # ---------------- phase 0: load pre-transposed x --------------
            _dma_engines = [nc.sync, nc.gpsimd, nc.scalar]
            for et in range(ET):
                _dma_engines[et % 3].dma_start(
                    xTs[et][:], xb[et * 128 : (et + 1) * 128, :]
                )

            # ---------------- phase 1: q' = xA (transposed), V + LN1 ------
            with tc.tile_pool(name="p1sb", bufs=1) as p1sb, \
                 tc.tile_pool(name="p1a", bufs=1) as p1a, \
                 tc.tile_pool(name="p1v", bufs=2) as p1v, \
                 tc.tile_pool(name="p1small", bufs=4) as p1small, \
                 tc.tile_pool(name="p1ps", bufs=4, space="PSUM") as p1ps, \
                 tc.tile_pool(name="p1psv", bufs=4, space="PSUM") as p1psv:
                # A slabs first (qp needs them immediately); wv afterwards
                # (V matmuls run only after the whole qp sweep)
                a_sbs = []
                for fb in range(ET):
                    a_sb = p1a.tile([128, ET, 128], BF16, name=f"a{fb}")
                    _dma_engines[fb % 3].dma_start(a_sb[:], a_w[fb])
                    a_sbs.append(a_sb)
                wv_sb = p1sb.tile([128, ET, E], BF16)
                for et in range(ET):
                    _dma_engines[(et + 1) % 3].dma_start(wv_sb[:, et, :], wv_w[:, et, :])

                # q'^T[f, s] — accumulate over e tiles
                for fb in range(ET):
                    a_sb = a_sbs[fb]
                    for sc in range(KC):
                        ps_q = p1ps.tile([128, 512], F32)
                        for et in range(ET):
                            nc.tensor.matmul(
                                ps_q[:],
                                a_sb[:, et, :],
                                xTs[et][:, sc * 512 : (sc + 1) * 512],
                                start=(et == 0),
                                stop=(et == ET - 1),
                            )
                        nc.scalar.copy(qT[:, fb, sc * 512 : (sc + 1) * 512], ps_q[:])

                # V[s, f] token-major; LN1 fused on evacuation
                for si in range(SB):
                    ps_v = []
                    for fc in range(EC):
                        pv = p1psv.tile([128, 512], F32)
                        ps_v.append(pv)
                        for et in range(ET):
                            nc.tensor.matmul(
                                pv[:],
                                xTs[et][:, si * 128 : (si + 1) * 128],
                                wv_sb[:, et, fc * 512 : (fc + 1) * 512],
                                start=(et == 0),
                                stop=(et == ET - 1),
                            )
                    v_sb = p1v.tile([128, E], F32)
                    for fc in range(EC):
                        nc.scalar.copy(v_sb[:, fc * 512 : (fc + 1) * 512], ps_v[fc][:])
                    if flags["has_vbias"]:
                        nc.vector.tensor_add(v_sb[:], v_sb[:], bv_sb[:])
                    # LN1 stats
                    st = p1small.tile([128, EC, 6], F32)
                    for fc in range(EC):
                        nc.vector.bn_stats(st[:, fc, :], v_sb[:, fc * 512 : (fc + 1) * 512])
                    mv = p1small.tile([128, 2], F32)
                    nc.vector.bn_aggr(mv[:], st[:])
                    lnv = p1small.tile([128, 1], F32)
                    nc.scalar.activation(lnv[:], mv[:, 1:2], Ln, bias=eps_t[:])
                    rstd = p1small.tile([128, 1], F32)
                    # rstd * vnscale  (vn holds LN1(V) * vnscale)
                    if vnscale == 8.0:
                        nc.scalar.activation(
                            rstd[:], lnv[:], Exp, scale=-0.5, bias=ln8_t[:]
                        )
                    else:
                        nc.scalar.activation(rstd[:], lnv[:], Exp, scale=-0.5)
                    nc.vector.tensor_scalar(
                        out=vn[:, si, :E], in0=v_sb[:], scalar1=mv[:, 0:1],
                        scalar2=rstd[:], op0=SUB, op1=MUL,
                    )
                    if flags["has_ln1_affine"]:
                        nc.vector.tensor_mul(vn[:, si, :E], vn[:, si, :E], g1_sb[:])
                        nc.vector.tensor_add(vn[:, si, :E], vn[:, si, :E], c1_sb[:])
                    nc.sync.dma_start(vscr[si], v_sb[:])

            # ---------------- phase 2: attention + LN2 (transposed scores) -
            # PE stream interleaves the score panel of query-chunk qc with the
            # ctx matmuls of qc-1 so the tensor engine never drains.  All ctx
            # PSUM is evacuated immediately via ACT copies to SBUF; the LN2
            # chain runs from SBUF off the PE critical path.
            with tc.tile_pool(name="p2pT", bufs=2) as p2pT, \
                 tc.tile_pool(name="p2small", bufs=6) as p2small, \
                 tc.tile_pool(name="p2u", bufs=2) as p2u, \
                 tc.tile_pool(name="p2h", bufs=2) as p2h, \
                 tc.tile_pool(name="psS", bufs=3, space="PSUM") as psS_pool, \
                 tc.tile_pool(name="psT", bufs=1, space="PSUM") as psT_pool, \
                 tc.tile_pool(name="psL", bufs=1, space="PSUM") as psL_pool, \
                 tc.tile_pool(name="psC", bufs=3, space="PSUM") as psC_pool:

                def scores_group(pT, qc, g):
                    """4 key blocks of the scores^T panel for query chunk qc."""
                    for kt in range(4 * g, 4 * g + 4):
                        ps_s = psS_pool.tile([128, 512], F32, tag="scT", name="ps_s")
                        for et in range(ET):
                            nc.tensor.matmul(
                                ps_s[:],
                                xTs[et][:, kt * 128 : (kt + 1) * 128],
                                qT[:, et, qc * 512 : (qc + 1) * 512],
                                start=(et == 0),
                                stop=(et == ET - 1),
                            )
                        nc.scalar.activation(
                            pT[:, kt, :], ps_s[:], Exp,
                            bias=cb_sb[:, kt : kt + 1] if flags["has_colbias"]
                            else expb[:],
                        )

                def ctx_block(pT, qc, qj):
                    """ctx + l matmuls and LN2 for query block qj of chunk qc."""
                    qi = qc * 4 + qj
                    ps_l = psL_pool.tile([128, 16], F32, tag="l", name="ps_l")
                    ps_c = [
                        psC_pool.tile([128, 512], F32, name="ps_c")
                        for _ in range(EC)
                    ]
                    if CTX_E4:
                        # kt-pair-major: stationary P slice reused across
                        # ec0 / ec1 / l matmuls
                        for kt in dr_pairs(SB):
                            stat = pT[:, kt : kt + 2, qj * 128 : (qj + 1) * 128]
                            for ec in range(EC):
                                nc.tensor.matmul(
                                    ps_c[ec][:], stat,
                                    vn[:, kt : kt + 2, ec * 512 : (ec + 1) * 512],
                                    perf_mode=mybir.MatmulPerfMode.DoubleRow,
                                    start=(kt == 0), stop=(kt == SB - 2),
                                )
                            nc.tensor.matmul(
                                ps_l[:], stat,
                                vn[:, kt : kt + 2, E:EE],
                                perf_mode=mybir.MatmulPerfMode.DoubleRow,
                                start=(kt == 0), stop=(kt == SB - 2),
                            )
                    else:
                        for kt in range(SB):
                            stat = pT[:, kt, qj * 128 : (qj + 1) * 128]
                            for ec in range(EC):
                                nc.tensor.matmul(
                                    ps_c[ec][:], stat,
                                    vn[:, kt, ec * 512 : (ec + 1) * 512],
                                    start=(kt == 0), stop=(kt == SB - 1),
                                )
                            nc.tensor.matmul(
                                ps_l[:], stat,
                                vn[:, kt, E:EE],
                                start=(kt == 0), stop=(kt == SB - 1),
                            )
                    # fast PSUM evacuation (ACT) -> SBUF
                    u_sb = p2u.tile([128, E], F32, tag="u")
                    l_sb = p2small.tile([128, 1], F32, tag="lsb")
                    nc.vector.tensor_copy(l_sb[:], ps_l[:, 0:1])
                    for ec in range(EC):
                        nc.vector.tensor_copy(u_sb[:, ec * 512 : (ec + 1) * 512], ps_c[ec][:])
                    # LN2 with softmax normalization folded in (exact):
                    # h = (u - mu_u) * rstd_c / l,
                    # rstd_c = 1/sqrt(var_u/l^2 + eps)
                    st2 = p2small.tile([128, EC, 6], F32, tag="st2")
                    for ec in range(EC):
                        nc.vector.bn_stats(
                            st2[:, ec, :], u_sb[:, ec * 512 : (ec + 1) * 512]
                        )
                    mv2 = p2small.tile([128, 2], F32, tag="mv2")
                    nc.vector.bn_aggr(mv2[:], st2[:])
                    sinv = p2small.tile([128, 1], F32, tag="sinv")
                    nc.vector.reciprocal(sinv[:], l_sb[:])
                    t1 = p2small.tile([128, 1], F32, tag="t1")
                    nc.vector.tensor_mul(t1[:], mv2[:, 1:2], sinv[:])
                    nc.vector.tensor_mul(t1[:], t1[:], sinv[:])
                    lnv2 = p2small.tile([128, 1], F32, tag="lnv2")
                    nc.scalar.activation(
                        lnv2[:], t1[:], Ln, bias=eps_t[:], scale=ln_scale
                    )
                    rstd2 = p2small.tile([128, 1], F32, tag="rstd2")
                    if rstd_bias != 0.0:
                        nc.scalar.activation(
                            rstd2[:], lnv2[:], Exp, scale=-0.5, bias=ln8_t[:]
                        )
                    else:
                        nc.scalar.activation(rstd2[:], lnv2[:], Exp, scale=-0.5)
                    fac = p2small.tile([128, 1], F32, tag="fac")
                    nc.vector.tensor_mul(fac[:], rstd2[:], sinv[:])
                    h_tok = p2h.tile([128, E], BF16, name="h_tok")
                    for ec in range(EC):
                        nc.vector.tensor_scalar(
                            out=h_tok[:, ec * 512 : (ec + 1) * 512],
                            in0=u_sb[:, ec * 512 : (ec + 1) * 512],
                            scalar1=mv2[:, 0:1], scalar2=fac[:],
                            op0=SUB, op1=MUL,
                        )
                    return h_tok, qi

                def h_transpose(h_tok, qi):
                    # transpose h into hT (deferred one group so the LN2 chain
                    # has finished and the PE never waits on h_tok)
                    for g in range(2):
                        ps_t2 = psT_pool.tile([128, 512], BF16, tag="pstr", name="ps_t2")
                        for j in range(4):
                            fb = 4 * g + j
                            nc.tensor.transpose(
                                ps_t2[:, j * 128 : (j + 1) * 128],
                                h_tok[:, fb * 128 : (fb + 1) * 128],
                                ident[:],
                            )
                        nc.vector.tensor_copy(
                            hT_parts[qi // 4][
                                :, 4 * g : 4 * g + 4,
                                (qi % 4) * 128 : (qi % 4 + 1) * 128,
                            ],
                            ps_t2[:].rearrange("p (a b) -> p a b", a=4),
                        )

                pT_tiles = [None, None]
                pending_h = None
                for qc in range(KC):
                    pT_tiles[qc % 2] = p2pT.tile([128, SB, 512], P_DT, tag="pT",
                                                 name="pT")
                    for g in range(4):
                        scores_group(pT_tiles[qc % 2], qc, g)
                        if pending_h is not None:
                            h_transpose(*pending_h)
                        if qc > 0:
                            pending_h = ctx_block(pT_tiles[(qc - 1) % 2], qc - 1, g)
                for g in range(4):
                    if pending_h is not None:
                        h_transpose(*pending_h)
                    pending_h = ctx_block(pT_tiles[(KC - 1) % 2], KC - 1, g)
                h_transpose(*pending_h)

            acts_xq_cm.__exit__(None, None, None)
            acts_vn_cm.__exit__(None, None, None)

            # ---------------- phase 3: FFN + residual ----------------
            with tc.tile_pool(name="p3h1", bufs=1) as p3h1, \
                 tc.tile_pool(name="p3w1", bufs=6) as p3w1, \
                 tc.tile_pool(name="p3w2", bufs=1) as p3w2, \
                 tc.tile_pool(name="p3o", bufs=3) as p3o, \
                 tc.tile_pool(name="p3v", bufs=1) as p3v, \
                 tc.tile_pool(name="psH", bufs=2, space="PSUM") as psH_pool, \
                 tc.tile_pool(name="psO", bufs=6, space="PSUM") as psO_pool:
                w2_sb = p3w2.tile([128, HT, E], BF16)
                for ht in range(HT):
                    nc.sync.dma_start(
                        w2_sb[:, ht, :], w2_w[ht * 128 : (ht + 1) * 128, :]
                    )
                for sc in range(KC):  # 4 chunks of 512 tokens
                    v_pre = p3v.tile([128, 4, E], F32, tag="vpre", name="v_pre")
                    for j in range(4):
                        nc.gpsimd.dma_start(v_pre[:, j, :], vscr[sc * 4 + j])
                    if fastpath and sc == 0:
                        h1T = h1T_boot
                    else:
                        h1T = p3h1.tile([128, HT, 512], BF16, tag="h1T")
                    for hb in range(HT):
                        if sc == 0 and hb < n_boot:
                            w1_slice = w1b_sb[:, hb, :, :]
                        else:
                            w1_sb = p3w1.tile([128, ET, 128], BF16)
                            nc.sync.dma_start(w1_sb[:], w1_w[hb])
                            w1_slice = w1_sb[:]
                        ps_h = psH_pool.tile([128, 512], F32)
                        for et in range(ET):
                            nc.tensor.matmul(
                                ps_h[:],
                                w1_slice[:, et, :],
                                hT_parts[sc][:, et, :],
                                start=(et == 0),
                                stop=(et == ET - 1),
                            )
                        nc.scalar.activation(
                            h1T[:, hb, :], ps_h[:], Relu, bias=b1_sb[:, hb : hb + 1]
                        )
                    # second FFN layer + residual for the 4 s-blocks in chunk
                    for ec in range(EC):
                        ps_o = [
                            psO_pool.tile([128, 512], F32, tag="pso", name="pso") for _ in range(4)
                        ]
                        for ht in range(HT):
                            for j in range(4):
                                nc.tensor.matmul(
                                    ps_o[j][:],
                                    h1T[:, ht, j * 128 : (j + 1) * 128],
                                    w2_sb[:, ht, ec * 512 : (ec + 1) * 512],
                                    start=(ht == 0),
                                    stop=(ht == HT - 1),
                                )
                        for j in range(4):
                            si = sc * 4 + j
                            if flags["has_b2"]:
                                nc.vector.tensor_add(
                                    ps_o[j][:], ps_o[j][:],
                                    b2_sb[:, ec * 512 : (ec + 1) * 512],
                                )
                            o_sb = p3o.tile([128, 512], F32)
                            nc.scalar.activation(o_sb[:], ps_o[j][:], Relu)
                            nc.vector.tensor_add(
                                o_sb[:], o_sb[:],
                                v_pre[:, j, ec * 512 : (ec + 1) * 512],
                            )
                            out_q = nc.gpsimd if (sc == KC - 1 and j % 2) else nc.sync
                            out_q.dma_start(
                                out_w[si * 128 : (si + 1) * 128, ec * 512 : (ec + 1) * 512],
                                o_sb[:],
                            )

    _fixup_waits(nc)
    return nc


# ---------------------------------------------------------------------------
# Host wrapper
# ---------------------------------------------------------------------------
def kernel(
    xembeddings, mask, Wq_w, Wq_b, Wk_w, Wk_b, Wv_w, Wv_b,
    ln1_g, ln1_b, ln2_g, ln2_b, W1, b1, W2, b2,
):
    global LAST_EXEC_TIME_NS, LAST_RESULTS
    _apply_patches()
    trace = bool(os.environ.get("BASS_TRACE"))
    if trace:
        _maybe_install_ntff_hook()

    x = np.asarray(xembeddings, dtype=np.float32)
    mask = np.asarray(mask)
    f64 = np.float64

    # host-side exact folds (float64)
    A = (np.asarray(Wq_w, f64) @ np.asarray(Wk_w, f64).T) / np.sqrt(E)
    W1f = np.asarray(ln2_g, f64)[:, None] * np.asarray(W1, f64)
    b1f = np.asarray(b1, f64) + np.asarray(ln2_b, f64) @ np.asarray(W1, f64)

    # column bias on scores from the query bias: (x @ (Wk @ bq)) / sqrt(E)
    colbias = (x.astype(f64) @ (np.asarray(Wk_w, f64) @ np.asarray(Wq_b, f64))) / np.sqrt(E)
    maskbias = np.where(np.asarray(mask, bool), 0.0, -1e30)  # [B, S]
    cb = colbias + maskbias  # [B, S]
    has_colbias = bool(np.any(cb != 0.0))

    bv = np.asarray(Wv_b, np.float32)
    has_vbias = bool(np.any(bv != 0.0))
    g1 = np.asarray(ln1_g, np.float32)
    c1 = np.asarray(ln1_b, np.float32)
    has_ln1_affine = bool(np.any(g1 != 1.0) or np.any(c1 != 0.0))
    b2f = np.asarray(b2, np.float32)
    has_b2 = bool(np.any(b2f != 0.0))

    flags = {
        "has_colbias": has_colbias,
        "has_vbias": has_vbias,
        "has_ln1_affine": has_ln1_affine,
        "has_b2": has_b2,
    }

    bf = ml_dtypes.bfloat16
    # weight layouts (see _build):
    #   a_w/wv_w: [128 e_p, ET, E_out]  (per-partition contiguous)
    a_h = (A.astype(np.float32).astype(bf).reshape(ET, 128, ET, 128).transpose(2, 1, 0, 3).copy())
    wv_h = (
        np.asarray(Wv_w, np.float32).astype(bf).reshape(ET, 128, E).transpose(1, 0, 2).copy()
    )
    #   w1_w: [HT, 128 e_p, ET, 128 f]
    w1_h = (
        W1f.astype(np.float32).astype(bf)
        .reshape(ET, 128, HT, 128).transpose(2, 1, 0, 3).copy()
    )
    w2_h = np.asarray(W2, np.float32).astype(bf).copy()
    b1_h = b1f.astype(np.float32).reshape(HT, 128).T.copy()

    nc = _build(flags)

    in_maps = []
    for b_i in range(B):
        m = {
            "xb": np.ascontiguousarray(x[b_i].T).astype(bf),
            "a_w": a_h,
            "wv_w": wv_h,
            "w1_w": w1_h,
            "w2_w": w2_h,
            "b1_w": b1_h,
        }
        if has_colbias:
            # exp bias panel, key-major [128, SB]
            cbp = (cb[b_i].astype(np.float64) + (LN_P - C_OFF)).astype(np.float32)
            np.clip(cbp, -1e30, None, out=cbp)
            m["cb_w"] = np.ascontiguousarray(cbp.reshape(SB, 128).T)
        if has_vbias:
            m["bv_w"] = bv.reshape(1, E)
        if has_ln1_affine:
            m["g1_w"] = g1.reshape(1, E)
            m["c1_w"] = (c1 * (8.0 if CTX_E4 else 1.0)).reshape(1, E)
        if has_b2:
            m["b2_w"] = b2f.reshape(1, E)
        in_maps.append(m)

    from concourse.bass_utils import run_bass_kernel_spmd

    res = run_bass_kernel_spmd(
        nc, in_maps, core_ids=list(range(B)), trace=trace
    )
    LAST_EXEC_TIME_NS = res.exec_time_ns
    LAST_RESULTS = res
    out = np.stack([res.results[i]["out"] for i in range(B)], axis=0)
    return out.astype(np.float32)
